# revision 37
# baseline (speedup 1.0000x reference)
"""Trainium2 Bass kernel for BertEmbedding segment-mean-pool + linear.

Reference computation (per sentence i):
    pooled[t, :] = mean_{s : word_ids[i,s]==t} hidden[i, s, :]   (0 if empty)
    pooled[t, :] = 0 where t >= token_lengths[i]
    out[i] = pooled @ W.T + b                                    [T, E]

Shapes: hidden [64, 512, 768] f32, word_ids [64, 512] i32 (sorted per
sentence), token_lengths [64] i32, W [512, 768] f32, b [512] f32
-> out [64, 256, 512] f32.

Strategy (v2, data-parallel over batch, 8 sentences/core, SPMD):
  - All device tensors bf16 (host converts; tolerance is 2e-2).
  - Mean fold: host precomputes rscale[s] = (wid[s] < len) / count[wid[s]].
    One-hot is built as oh[s,t] = (wid[s]==t) * rscale[s] in one DVE
    tensor_scalar (is_equal then mult, both per-partition scalars), so
    mm1 produces pooled^T directly (out partitions = h-chunk).
  - Bias applied on HOST after gather (b is constant across rows).
  - len exploitation, SPMD-safe: sentences sorted by len desc and dealt
    round-robin, so slot s has similar L/K on every core. Only the K_s
    kept s-tiles are DMA'd (2 DMAs/slot: full tiles + partial rows),
    alternating the two HWDGE queues (SP/Activation).
  - mm1full: the first s-tile sweeps the full [0, L) with start=True so
    every later tile needs a single accumulate piece (132 matmuls/rep
    instead of 210; measured faster on HW despite extra rows).
  - mm2 batched over slot GROUPS with sum(L) <= 512 (one PSUM bank):
    pooled^T for a whole group is packed into one SBUF tile [128,6,G];
    mm2 is (4 e-chunks x 6 k) matmuls per group (72/rep vs 192), each
    streaming G columns; out is packed e-chunk-major into [128, 4G]
    bf16 and written with ONE full-rate DMA per group (3/rep).
  - Engine split (HW-measured): one-hot on DVE (GPSIMD is far slower
    per launch and per element on real HW), PSUM->SBUF copies split
    ACT/DVE/DVE, out copies alternate DVE/ACT.
  - Host gather: transpose [128e, 4ec, L] -> [L, 512] per sentence.
"""

import sys

if "/opt/trn_rl_repo" not in sys.path:
    sys.path.insert(0, "/opt/trn_rl_repo")

import numpy as np

B, S, H, E, T = 64, 512, 768, 512, 256
NCORES = 8
BL = B // NCORES  # sentences per core
KS = S // 128  # max s-tiles
KH = H // 128  # 6 h-chunks (contraction of matmul 2)
CT = T // 128  # max t-chunks of the output

_cache: dict = {}
_PARAMS = None  # (L_s, K_s) per slot, set by _set_params from input data


def _set_params(word_ids, token_lengths):
    """Slot assignment + per-slot static bounds from the actual inputs.

    Per slot: L = max len, K = max s-tiles, and per-tile column regions
    [a_k, b_k) with fresh-start points f_k. Region k covers every word id
    that any core's s-tile k contains (below its len); f_k = max(a_k,
    max_{j<k} b_j) so each column gets start=True from exactly its first
    toucher and start=False from later ones.

    Also packs slots into groups with sum(L) <= 512 (one PSUM bank) for
    the batched-mm2 path: groups = ((slot_ids, off, G, (o_s,...)), ...).
    """
    global _PARAMS
    wid = np.asarray(word_ids)
    lens = np.asarray(token_lengths).astype(np.int64)
    order = np.argsort(-lens, kind="stable")  # sentences by len desc
    slots = []
    for s in range(BL):
        grp = order[NCORES * s : NCORES * (s + 1)]
        L = int(lens[grp].max())
        Sk = [int(np.searchsorted(wid[i], lens[i])) for i in grp]
        Ks = [max(1, (s_ + 127) // 128) for s_ in Sk]
        K = max(Ks)
        P = min(128, max(1, max(Sk) - 128 * (K - 1)))
        regions = []
        prev_end = 0
        for k in range(K):
            lo, hi = T, 0
            for i, Ki, skeep in zip(grp, Ks, Sk):
                if Ki <= k:
                    continue
                lo = min(lo, int(wid[i][128 * k]))
                hi = max(hi, int(wid[i][min(128 * k + 127, skeep - 1)]) + 1)
            a = min(max(0, lo), prev_end) if k > 0 else 0
            b = min(max(hi, a), L) if k < K - 1 else L
            f = max(a, prev_end)
            regions.append((a, b, f))
            prev_end = max(prev_end, b)
        slots.append((L, K, P, tuple(regions)))

    groups, cur, cum = [], [], 0
    off = 0
    for s in range(BL):
        L = slots[s][0]
        if cur and cum + L > 512:
            groups.append((tuple(cur), off, cum, tuple(np.cumsum([0] + [slots[x][0] for x in cur])[:-1])))
            off += cum
            cur, cum = [], 0
        cur.append(s)
        cum += L
    groups.append((tuple(cur), off, cum, tuple(np.cumsum([0] + [slots[x][0] for x in cur])[:-1])))
    _PARAMS = (tuple(slots), tuple(int(x) for x in order), tuple(groups))
    return _PARAMS


def _build(reps: int = 1, bufs: int = 4, ptseng: str = "svs", outeng: str = "vs",
           dmaq: int = 2, mm2kout: int = 0, oheng: str = "g", nobands: int = 0, mm2rev: int = 0, wsrep: int = 0, dmaonly: int = 0, outdma: int = 0, slotiv: int = 0, dbg: int = 0, ablate: str = ""):
    """Build + compile the per-core Bass program for the current _PARAMS.

    ptseng: 3 chars, engines for the 3 pooled PSUM->SBUF copies
    outeng: engines for out chunk copies (cycled)
    chars: s=scalar(ACT), v=vector(DVE), g=gpsimd(Pool)
    """
    assert _PARAMS is not None, "_set_params must run before _build"
    slots = _PARAMS[0]
    ablated = set(ablate.split(",")) if ablate else set()
    from concourse import bacc, tile, mybir

    f32 = mybir.dt.float32
    bf16 = mybir.dt.bfloat16
    i32 = mybir.dt.int32
    Alu = mybir.AluOpType

    nc = bacc.Bacc("TRN2", target_bir_lowering=False, debug=False, num_devices=NCORES)

    def eng(ch):
        return {"s": nc.scalar, "v": nc.vector, "g": nc.gpsimd}[ch]

    h_d = nc.dram_tensor("h", [BL, 128, KS, H], bf16, kind="ExternalInput")
    if dbg:
        dbg_oh = nc.dram_tensor("dbg_oh", [128, KS, T], bf16, kind="ExternalOutput")
        dbg_pts = nc.dram_tensor("dbg_pts", [128, KH, T], bf16, kind="ExternalOutput")
        dbg_hs = nc.dram_tensor("dbg_hs", [128, KS, H], bf16, kind="ExternalOutput")
    ws_d = nc.dram_tensor("ws", [BL, 128, 2 * KS], f32, kind="ExternalInput")
    if wsrep:
        wsr_d = nc.dram_tensor("wsr", [128, BL, 2 * KS], f32, kind="ExternalInput")
    wt_d = nc.dram_tensor("wt", [H, E], bf16, kind="ExternalInput")  # W^T
    out_d = nc.dram_tensor("out", [BL, T, E], bf16, kind="ExternalOutput")
    if mm2rev == 2:
        outT_d = nc.dram_tensor("outT", [BL, 128, 4, T], bf16, kind="ExternalOutput")

    with tile.TileContext(nc) as tc:
        with (
            tc.tile_pool(name="const", bufs=1) as cpool,
            tc.tile_pool(name="work", bufs=bufs) as wpool,
            tc.tile_pool(name="pp", bufs=2, space="PSUM") as ppool,
            tc.tile_pool(name="po", bufs=1, space="PSUM") as opool,
        ):
            # ---- one-time constants ----
            iota_i = cpool.tile([128, T], i32)
            nc.gpsimd.iota(iota_i[:], pattern=[[1, T]], base=0, channel_multiplier=0)
            iota_b = cpool.tile([128, T], bf16)
            nc.vector.tensor_copy(iota_b[:], iota_i[:])
            wt_t = cpool.tile([128, KH, E], bf16)
            nc.sync.dma_start(wt_t[:], wt_d[:, :].rearrange("(k p) e -> p k e", p=128))

            state = {}
            wsr_t = {"t": None}

            def stage_a(it):
                """Load + one-hot + mm1 for sentence slot it%BL."""
                i = it % BL
                L, K, P, regions = slots[i]
                hs = wpool.tile([128, KS, H], bf16, tag="hs")
                h_src = h_d[i]
                full = K - 1  # tiles loaded with all 128 rows
                if dmaq >= 2 and full >= 1:
                    k2 = (full + 1) // 2
                    nc.sync.dma_start(hs[:, 0:k2, :], h_src[:, 0:k2, :])
                    if full > k2:
                        nc.scalar.dma_start(hs[:, k2:full, :], h_src[:, k2:full, :])
                    nc.scalar.dma_start(hs[0:P, full, :], h_src[0:P, full, :])
                else:
                    if full >= 1:
                        nc.sync.dma_start(hs[:, 0:full, :], h_src[:, 0:full, :])
                    nc.sync.dma_start(hs[0:P, full, :], h_src[0:P, full, :])
                if wsrep:
                    if i == 0:
                        wsr_t["t"] = wpool.tile([128, BL, 2 * KS], f32, name="wsr", tag="wsr")
                        nc.sync.dma_start(wsr_t["t"][:], wsr_d[:])
                    ws_t = wsr_t["t"][:, i, :]
                else:
                    ws_tile = wpool.tile([128, 2 * KS], f32, tag="ws")
                    nc.gpsimd.dma_start(ws_tile[:], ws_d[i])
                    ws_t = ws_tile[:]

                if dmaonly:
                    state[it] = None
                    return
                # scaled one-hot oh[s, t] = (wid==t) * rscale
                oh = wpool.tile([128, KS, T], bf16, tag="oh")
                if dbg and it == 0:
                    nc.gpsimd.memset(oh[:], 0.0)
                ohrg = [(0, L, 0)] * K if nobands else regions
                for k, (a, b, f) in enumerate(ohrg):
                    pk = P if k == K - 1 else 128
                    if b > a:
                        eng(oheng).tensor_scalar(
                            oh[0:pk, k, a:b],
                            iota_b[0:pk, a:b],
                            ws_t[0:pk, k : k + 1],
                            ws_t[0:pk, KS + k : KS + k + 1],
                            Alu.is_equal,
                            Alu.mult,
                        )

                # matmul 1: pooled^T [h, t] (already mean-scaled).
                # Region k writes cols [a,b): [a,f) accumulates onto earlier
                # tiles (start=False), [f,b) is this tile's fresh range
                # (start=True). Every column is started exactly once.
                pt_ps = [
                    ppool.tile([128, 2, T], f32, name=f"pt{j}", tag=f"pt{j}")
                    for j in range(3)
                ]
                for m in range(KH if "mm1" not in ablated else 0):
                    if nobands:
                        for k in range(K):
                            pk = P if k == K - 1 else 128
                            nc.tensor.matmul(
                                pt_ps[m // 2][:, m % 2, 0:L],
                                hs[0:pk, k, m * 128 : (m + 1) * 128],
                                oh[0:pk, k, 0:L],
                                start=(k == 0),
                                stop=(k == K - 1),
                            )
                        continue
                    for k, (a, b, f) in enumerate(regions):
                        last = k == K - 1
                        pk = P if k == K - 1 else 128
                        if f > a:
                            nc.tensor.matmul(
                                pt_ps[m // 2][:, m % 2, a:f],
                                hs[0:pk, k, m * 128 : (m + 1) * 128],
                                oh[0:pk, k, a:f],
                                start=False,
                                stop=last,
                                skip_group_check=True,
                            )
                        if b > f:
                            nc.tensor.matmul(
                                pt_ps[m // 2][:, m % 2, f:b],
                                hs[0:pk, k, m * 128 : (m + 1) * 128],
                                oh[0:pk, k, f:b],
                                start=True,
                                stop=last,
                                skip_group_check=True,
                            )
                if "mm1" in ablated:
                    for j in range(3):
                        nc.vector.memset(pt_ps[j][:], 0.5)
                if dbg and it == 0:
                    nc.sync.dma_start(dbg_oh[:], oh[:])
                    nc.sync.dma_start(dbg_hs[:], hs[:])
                state[it] = pt_ps

            def stage_b(it):
                """pooled->SBUF, mm2, out copy + DMA for sentence slot it%BL."""
                i = it % BL
                L, K, P, regions = slots[i]
                CH = (L + 127) // 128
                C = 128 * CH
                pt_ps = state.pop(it)
                if dmaonly:
                    return

                pts = wpool.tile([128, KH, T], bf16, tag="pts")
                if L < C and mm2rev != 2:
                    nc.gpsimd.memset(pts[:, :, L:C], 0.0)
                for j in range(3):
                    dst = pts[:, 2 * j : 2 * j + 2, 0:L]
                    src = pt_ps[j][:, :, 0:L]
                    ech = ptseng[j % len(ptseng)]
                    if ech == "s":
                        nc.scalar.copy(dst, src)
                    else:
                        eng(ech).tensor_copy(dst, src)

                if dbg and it == 0:
                    nc.sync.dma_start(dbg_pts[:], pts[:])

                if mm2rev == 2:
                    # reversed mm2, e-major straight to DRAM (host transposes)
                    oT = [
                        opool.tile([128, 2, T], f32, name=f"oT{j}", tag=f"oT{j}")
                        for j in range(2)
                    ]
                    for j in range(2):
                        for ec in range(2):
                            e0 = (2 * j + ec) * 128
                            for k in range(KH):
                                nc.tensor.matmul(
                                    oT[j][:, ec, 0:L],
                                    wt_t[:, k, e0 : e0 + 128],
                                    pts[:, k, 0:L],
                                    start=(k == 0),
                                    stop=(k == KH - 1),
                                )
                    outsT = wpool.tile([128, 4, T], bf16, tag="outsT")
                    for j in range(2):
                        ech = outeng[j % len(outeng)]
                        dstT = outsT[:, 2 * j : 2 * j + 2, 0:L]
                        srcT = oT[j][:, :, 0:L]
                        if ech == "s":
                            nc.scalar.copy(dstT, srcT)
                        else:
                            eng(ech).tensor_copy(dstT, srcT)
                        if outdma == 0:
                            dq = nc.sync if j == 0 else nc.scalar
                            dq.dma_start(
                                outT_d[i, :, 2 * j : 2 * j + 2, 0:L],
                                outsT[:, 2 * j : 2 * j + 2, 0:L],
                            )
                    if outdma == 1:
                        nc.gpsimd.dma_start(
                            outT_d[i, :, :, 0:L], outsT[:, :, 0:L]
                        )
                    elif outdma == 3:
                        dq = nc.sync if i % 2 else nc.scalar
                        dq.dma_start(outT_d[i, :, :, 0:L], outsT[:, :, 0:L])
                    elif outdma == 2:
                        nc.gpsimd.dma_start(
                            outT_d[i, :, 0:2, 0:L], outsT[:, 0:2, 0:L]
                        )
                        nc.vector.dma_start(
                            outT_d[i, :, 2:4, 0:L], outsT[:, 2:4, 0:L]
                        )
                    return

                if mm2rev:
                    # matmul 2 reversed: oT[e, t] = W^T-chunk.T @ pooled^T,
                    # streaming only L moving cols; transpose back to [t, e]
                    # via the DMA xbar (16x128 tiles, needs t % 128 == 0).
                    oT = [
                        opool.tile([128, 2, T], f32, name=f"oT{j}", tag=f"oT{j}")
                        for j in range(2)
                    ]
                    for j in range(2):
                        for ec in range(2):
                            e0 = (2 * j + ec) * 128
                            for k in range(KH):
                                nc.tensor.matmul(
                                    oT[j][:, ec, 0:L],
                                    wt_t[:, k, e0 : e0 + 128],
                                    pts[:, k, 0:L],
                                    start=(k == 0),
                                    stop=(k == KH - 1),
                                )
                    outsT = wpool.tile([128, 4, T], bf16, tag="outsT")
                    if L < C:
                        nc.gpsimd.memset(outsT[:, :, L:C], 0.0)
                    for j in range(2):
                        ech = ptseng[j % len(ptseng)]
                        dstT = outsT[:, 2 * j : 2 * j + 2, 0:L]
                        srcT = oT[j][:, :, 0:L]
                        if ech == "s":
                            nc.scalar.copy(dstT, srcT)
                        else:
                            eng(ech).tensor_copy(dstT, srcT)
                    outs = wpool.tile([128, CT, E], bf16, tag="outs")
                    for c in range(CH):
                        for ec in range(4):
                            dq = nc.sync if ((c * 4 + ec) % 2 == 0) else nc.scalar
                            dq.dma_start_transpose(
                                outs[:, c, ec * 128 : (ec + 1) * 128],
                                outsT[:, ec, c * 128 : (c + 1) * 128],
                            )
                        dq = nc.sync if (c % 2 == 0) else nc.scalar
                        dq.dma_start(out_d[i, c * 128 : (c + 1) * 128, :], outs[:, c, :])
                    return

                # matmul 2: out[t, e] = pooled @ W^T (k-outer so each pts
                # copy unblocks its accumulation step immediately)
                out_ps = [
                    opool.tile([128, E], f32, name=f"o2{c}", tag=f"o2{c}")
                    for c in range(CH)
                ]
                mm2iv = "mm2" not in ablated
                if not mm2iv:
                    for c in range(CH):
                        nc.vector.memset(out_ps[c][:], 0.25)
                order = (
                    [(k, c) for k in range(KH) for c in range(CH)]
                    if mm2kout
                    else [(k, c) for c in range(CH) for k in range(KH)]
                )
                for k, c in order if mm2iv else []:
                    nc.tensor.matmul(
                        out_ps[c][:],
                        pts[:, k, c * 128 : (c + 1) * 128],
                        wt_t[:, k, :],
                        start=(k == 0),
                        stop=(k == KH - 1),
                    )

                outs = wpool.tile([128, CT, E], bf16, tag="outs")
                for c in range(CH):
                    ech = outeng[c % len(outeng)]
                    if ech == "s":
                        nc.scalar.copy(outs[:, c, :], out_ps[c][:])
                    else:
                        eng(ech).tensor_copy(outs[:, c, :], out_ps[c][:])
                    dq = nc.sync if (c % 2 == 0) else nc.scalar
                    dq.dma_start(out_d[i, c * 128 : (c + 1) * 128, :], outs[:, c, :])

            if slotiv:
                seq = []
                lo, hi = 0, BL - 1
                while lo <= hi:
                    seq.append(lo)
                    if hi != lo:
                        seq.append(hi)
                    lo, hi = lo + 1, hi - 1
            else:
                seq = list(range(BL))

            def slot_of(it):
                return seq[it % BL]

            n = BL * reps
            stage_a(slot_of(0))
            for it in range(n):
                if it + 1 < n:
                    stage_a(slot_of(it + 1))
                stage_b(slot_of(it))

    nc.compile()
    return nc


def _build_v2(reps: int = 1, bufs: int = 8, ptseng: str = "svv", outeng: str = "vs",
              oheng: str = "g", mm1full: int = 0, gpbufs: int = 3, wseng: str = "g",
              hwloop: int = 0, ablate: str = "", hsmode: int = 0, hsq: int = 2,
              gmm1: int = 0, outq: int = 0):
    """Grouped-mm2 builder: fewer DMAs, 72 mm2 matmuls, packed out DMA.

    - hidden: one DMA for full s-tiles + one for the partial tile (2/slot).
    - mm2 batched over slot groups with sum(L) <= 512: pooled^T for a whole
      group lives in one SBUF tile [128, 6, G]; mm2 runs (4 e-chunks x 6 k)
      per group into a single-bank PSUM tile [128, G].
    - out: per group, 4 e-chunk copies pack into [128, 4G] bf16, one DMA.
    - mm1full=1: first s-tile sweeps [0, L) fresh so later tiles need one
      accumulate piece each (fewer PE instructions, slightly more rows).
    """
    assert _PARAMS is not None, "_set_params must run before _build_v2"
    slots, _, groups = _PARAMS
    TOT = sum(g[2] for g in groups)
    abl = set(ablate.split(",")) if ablate else set()
    from concourse import bacc, tile, mybir

    f32 = mybir.dt.float32
    bf16 = mybir.dt.bfloat16
    i32 = mybir.dt.int32
    Alu = mybir.AluOpType

    nc = bacc.Bacc("TRN2", target_bir_lowering=False, debug=False, num_devices=NCORES)

    def eng(ch):
        return {"s": nc.scalar, "v": nc.vector, "g": nc.gpsimd}[ch]

    def copy_to(ch, dst, src):
        if ch == "s":
            nc.scalar.copy(dst, src)
        else:
            eng(ch).tensor_copy(dst, src)

    h_d = nc.dram_tensor("h", [BL, 128, KS, H], bf16, kind="ExternalInput")
    ws_d = nc.dram_tensor("ws", [BL, 128, 2 * KS], f32, kind="ExternalInput")
    wsr_d = nc.dram_tensor("wsr", [128, BL, 2 * KS], f32, kind="ExternalInput")
    wt_d = nc.dram_tensor("wt", [H, E], bf16, kind="ExternalInput")  # W^T
    outP_d = nc.dram_tensor("outP", [128, 4 * TOT], bf16, kind="ExternalOutput")

    with tile.TileContext(nc) as tc:
        with (
            tc.tile_pool(name="const", bufs=1) as cpool,
            tc.tile_pool(name="work", bufs=bufs) as wpool,
            tc.tile_pool(name="grp", bufs=gpbufs) as gpool,
            tc.tile_pool(name="pp", bufs=(1 if gmm1 else 2), space="PSUM") as ppool,
            tc.tile_pool(name="po", bufs=1, space="PSUM") as opool,
        ):
            # ---- one-time constants ----
            iota_i = cpool.tile([128, T], i32)
            nc.gpsimd.iota(iota_i[:], pattern=[[1, T]], base=0, channel_multiplier=0)
            iota_b = cpool.tile([128, T], bf16)
            nc.vector.tensor_copy(iota_b[:], iota_i[:])
            wt_t = cpool.tile([128, KH, E], bf16)
            nc.sync.dma_start(wt_t[:], wt_d[:, :].rearrange("(k p) e -> p k e", p=128))

            state: dict = {}

            if "mm2only" in abl:
                # PE p-state probe: resident pooled tiles, mm2 stream only.
                for gi in range(len(groups)):
                    t = cpool.tile([128, KH, 512], bf16, name=f"ptsc{gi}", tag=f"ptsc{gi}")
                    nc.vector.memset(t[:], 0.25)
                    state[("ptsc", gi)] = t

            def stage_a(i, gi, o_s, G, hq, last_in_group=False):
                """Load + one-hot + mm1 + pts copy for slot i (group gi)."""
                if "mm2only" in abl:
                    return
                L, K, P, regions = slots[i]
                hs = wpool.tile([128, KS, H], bf16, tag="hs")
                h_src = h_d[i]
                hq_ring = {2: [nc.sync, nc.scalar],
                           3: [nc.sync, nc.scalar, nc.gpsimd],
                           4: [nc.sync, nc.scalar, nc.vector]}[hsq]
                dq = hq_ring[hq % len(hq_ring)]
                dq2 = hq_ring[(hq + 1) % len(hq_ring)]
                if "hsdma" not in abl:
                    if hsmode == 2:
                        dq.dma_start(hs[:, :, :], h_src[:, :, :])
                    elif hsmode == 1:
                        dq.dma_start(hs[:, 0:K, :], h_src[:, 0:K, :])
                    else:
                        if K > 1:
                            dq.dma_start(hs[:, 0 : K - 1, :], h_src[:, 0 : K - 1, :])
                        dq2.dma_start(hs[0:P, K - 1, :], h_src[0:P, K - 1, :])
                if i == 0:
                    state["wsr"] = wpool.tile([128, BL, 2 * KS], f32, name="wsr", tag="wsr")
                    if "wsdma" not in abl:
                        eng(wseng).dma_start(state["wsr"][:], wsr_d[:])
                ws_t = state["wsr"][:, i, :]

                # scaled one-hot oh[s, t] = (wid==t) * rscale
                oh = wpool.tile([128, KS, T], bf16, tag="oh")
                ohrg = [((0, L, 0) if (mm1full and k == 0) else r)
                        for k, r in enumerate(regions)]
                for k, (a, b, f) in enumerate(ohrg):
                    pk = P if k == K - 1 else 128
                    if b > a and "oh" not in abl:
                        eng(oheng).tensor_scalar(
                            oh[0:pk, k, a:b],
                            iota_b[0:pk, a:b],
                            ws_t[0:pk, k : k + 1],
                            ws_t[0:pk, KS + k : KS + k + 1],
                            Alu.is_equal,
                            Alu.mult,
                        )

                # matmul 1: pooled^T [h, t] (mean-scaled via rscale)
                if o_s == 0:
                    state[("pts", gi)] = gpool.tile([128, KH, 512], bf16, name="ptsg", tag="ptsg")
                pts_g = state[("pts", gi)]

                if gmm1:
                    # accumulate straight into the per-group PSUM tile
                    if o_s == 0:
                        state[("ptG", gi)] = ppool.tile(
                            [128, KH, 512], f32, name="ptG", tag="ptG")
                    ptg = state[("ptG", gi)]

                    def mm1_out(m, c0, c1):
                        return ptg[:, m, o_s + c0 : o_s + c1]
                else:
                    pt_ps = [
                        ppool.tile([128, 2, T], f32, name=f"pt{j}", tag=f"pt{j}")
                        for j in range(3)
                    ]

                    def mm1_out(m, c0, c1):
                        return pt_ps[m // 2][:, m % 2, c0:c1]

                for m in range(KH if "mm1" not in abl else 0):
                    for k, (a, b, f) in enumerate(ohrg):
                        last = k == K - 1
                        pk = P if k == K - 1 else 128
                        if mm1full:
                            nc.tensor.matmul(
                                mm1_out(m, a, b),
                                hs[0:pk, k, m * 128 : (m + 1) * 128],
                                oh[0:pk, k, a:b],
                                start=(k == 0),
                                stop=last,
                                skip_group_check=True,
                            )
                            continue
                        if f > a:
                            nc.tensor.matmul(
                                mm1_out(m, a, f),
                                hs[0:pk, k, m * 128 : (m + 1) * 128],
                                oh[0:pk, k, a:f],
                                start=False,
                                stop=last,
                                skip_group_check=True,
                            )
                        if b > f:
                            nc.tensor.matmul(
                                mm1_out(m, f, b),
                                hs[0:pk, k, m * 128 : (m + 1) * 128],
                                oh[0:pk, k, f:b],
                                start=True,
                                stop=last,
                                skip_group_check=True,
                            )
                    if (gmm1 and last_in_group and m % 2 == 1
                            and "ptscopy" not in abl and "mm1" not in abl):
                        j = m // 2
                        copy_to(ptseng[j % len(ptseng)],
                                pts_g[:, 2 * j : 2 * j + 2, 0:G],
                                state[("ptG", gi)][:, 2 * j : 2 * j + 2, 0:G])

                # pooled^T -> group SBUF tile (f32 -> bf16), 3 copies
                if not gmm1:
                    for j in range(3 if "ptscopy" not in abl else 0):
                        copy_to(ptseng[j % len(ptseng)],
                                pts_g[:, 2 * j : 2 * j + 2, o_s : o_s + L],
                                pt_ps[j][:, :, 0:L])

            def stage_b(gi, off, G, oq):
                """Batched mm2 + packed out copy + one DMA for group gi."""
                if "mm2only" in abl:
                    pts_g = state[("ptsc", gi)]
                else:
                    pts_g = state.pop(("pts", gi))
                outs = gpool.tile([128, 4 * 512], bf16, tag="outsg")
                for ec in range(4):
                    oT = opool.tile([128, 512], f32, name=f"oT{ec % 2}", tag=f"oT{ec % 2}")
                    for k in range(KH if "mm2" not in abl else 0):
                        nc.tensor.matmul(
                            oT[:, 0:G],
                            wt_t[:, k, ec * 128 : (ec + 1) * 128],
                            pts_g[:, k, 0:G],
                            start=(k == 0),
                            stop=(k == KH - 1),
                        )
                    if "outcopy" not in abl:
                        copy_to(outeng[ec % len(outeng)],
                                outs[:, ec * G : (ec + 1) * G], oT[:, 0:G])
                if "outdma" not in abl:
                    if outq == 1:
                        dq = nc.gpsimd
                    elif outq == 2:
                        dq = nc.vector
                    else:
                        dq = nc.sync if oq % 2 == 0 else nc.scalar
                    dq.dma_start(outP_d[:, 4 * off : 4 * off + 4 * G], outs[:, 0 : 4 * G])

            NG = len(groups)
            qs = {"hq": 0, "oq": 0}

            def emit_reps():
                for r in range(reps):
                    for gi, (gslots, off, G, offs) in enumerate(groups):
                        for si, s in enumerate(gslots):
                            stage_a(s, gi, int(offs[si]), G, qs["hq"],
                                    last_in_group=(si == len(gslots) - 1))
                            qs["hq"] += 1
                        if gi >= 1:
                            pg = groups[gi - 1]
                            stage_b(gi - 1, pg[1], pg[2], qs["oq"])
                            qs["oq"] += 1
                    lg = groups[NG - 1]
                    stage_b(NG - 1, lg[1], lg[2], qs["oq"])
                    qs["oq"] += 1

            if hwloop:
                with tc.For_i(0, hwloop) as _i:
                    emit_reps()
            else:
                emit_reps()

    nc.compile()
    return nc


def _get_nc(reps: int = 1, **opts):
    key = f"nc{reps}|{sorted(opts.items())}|{hash(_PARAMS)}"
    if key not in _cache:
        o = dict(opts)
        if o.pop("v2", 0):
            _cache[key] = _build_v2(reps, **o)
        else:
            _cache[key] = _build(reps, **o)
    return _cache[key]


def _in_maps(hidden_states, word_ids, token_lengths, W, b):
    """Per-core input maps (also sets _PARAMS / slot assignment)."""
    import ml_dtypes

    bf16 = ml_dtypes.bfloat16
    if _PARAMS is None:
        _set_params(word_ids, token_lengths)
    order = _PARAMS[1]
    wid = np.asarray(word_ids)
    lens = np.asarray(token_lengths)

    wt = np.ascontiguousarray(W.T).astype(bf16)
    maps = [dict(wt=wt) for _ in range(NCORES)]
    h_all = np.empty((NCORES, BL, 128, KS, H), dtype=bf16)
    ws_all = np.zeros((NCORES, BL, 128, 2 * KS), dtype=np.float32)
    for s in range(BL):
        for c in range(NCORES):
            i = order[NCORES * s + c]
            h_all[c, s] = (
                hidden_states[i].reshape(KS, 128, H).transpose(1, 0, 2).astype(bf16)
            )
            cnt = np.bincount(wid[i], minlength=T).astype(np.float32)
            rs = np.where(wid[i] < lens[i], 1.0 / np.maximum(cnt[wid[i]], 1.0), 0.0)
            ws_all[c, s, :, 0:KS] = wid[i].reshape(KS, 128).T
            ws_all[c, s, :, KS : 2 * KS] = rs.reshape(KS, 128).T
    for c in range(NCORES):
        maps[c]["h"] = np.ascontiguousarray(h_all[c])
        maps[c]["ws"] = np.ascontiguousarray(ws_all[c])
        maps[c]["wsr"] = np.ascontiguousarray(ws_all[c].transpose(1, 0, 2))
    return maps


DEFAULT_OPTS: dict = {"v2": 1, "oheng": "v", "mm1full": 1}


def kernel(hidden_states, word_ids, token_lengths, W, b):
    from concourse import bass_utils

    _set_params(word_ids, token_lengths)
    maps = _in_maps(hidden_states, word_ids, token_lengths, W, b)
    nc = _get_nc(1, **DEFAULT_OPTS)
    res = bass_utils.run_bass_kernel_spmd(nc, maps, core_ids=list(range(NCORES)))

    slots, order = _PARAMS[0], _PARAMS[1]
    out = np.empty((B, T, E), dtype=np.float32)
    bia = b.astype(np.float32)[None, :]
    if DEFAULT_OPTS.get("v2"):
        groups = _PARAMS[2]
        for c in range(NCORES):
            outP = np.asarray(res.results[c]["outP"]).astype(np.float32)
            for gslots, off, G, offs in groups:
                arr = outP[:, 4 * off : 4 * off + 4 * G].reshape(128, 4, G)
                for si, s in enumerate(gslots):
                    L = slots[s][0]
                    o = int(offs[si])
                    i = order[NCORES * s + c]
                    blk = np.zeros((T, E), np.float32)
                    blk[:L] = arr[:, :, o : o + L].transpose(2, 1, 0).reshape(L, E)
                    blk += bia
                    out[i] = blk
        return out
    rev2 = DEFAULT_OPTS.get("mm2rev") == 2
    for s in range(BL):
        L = slots[s][0]
        CH = (L + 127) // 128
        for c in range(NCORES):
            if rev2:
                i = order[NCORES * s + c]
                oT = np.asarray(res.results[c]["outT"][s]).astype(np.float32)
                blk = np.zeros((T, E), np.float32)
                blk[:L] = oT.transpose(2, 1, 0).reshape(T, E)[:L]
                blk += bia
                out[i] = blk
                continue
            i = order[NCORES * s + c]
            blk = np.asarray(res.results[c]["out"][s]).astype(np.float32)
            blk[: 128 * CH] += bia
            blk[128 * CH :] = bia  # rows never touched on device
            out[i] = blk
    return out



# revision 38
# speedup vs baseline: 1.0217x; 1.0217x over previous
"""Trainium2 Bass kernel for BertEmbedding segment-mean-pool + linear.

Reference computation (per sentence i):
    pooled[t, :] = mean_{s : word_ids[i,s]==t} hidden[i, s, :]   (0 if empty)
    pooled[t, :] = 0 where t >= token_lengths[i]
    out[i] = pooled @ W.T + b                                    [T, E]

Shapes: hidden [64, 512, 768] f32, word_ids [64, 512] i32 (sorted per
sentence), token_lengths [64] i32, W [512, 768] f32, b [512] f32
-> out [64, 256, 512] f32.

Strategy (v2, data-parallel over batch, 8 sentences/core, SPMD):
  - All device tensors bf16 (host converts; tolerance is 2e-2).
  - Mean fold: host precomputes rscale[s] = (wid[s] < len) / count[wid[s]].
    One-hot is built as oh[s,t] = (wid[s]==t) * rscale[s] in one DVE
    tensor_scalar (is_equal then mult, both per-partition scalars), so
    mm1 produces pooled^T directly (out partitions = h-chunk).
  - Bias applied on HOST after gather (b is constant across rows).
  - len exploitation, SPMD-safe: sentences sorted by len desc and dealt
    round-robin, so slot s has similar L/K on every core. Only the K_s
    kept s-tiles are DMA'd (2 DMAs/slot: full tiles + partial rows),
    alternating the two HWDGE queues (SP/Activation).
  - mm1full: the first s-tile sweeps the full [0, L) with start=True so
    every later tile needs a single accumulate piece (132 matmuls/rep
    instead of 210; measured faster on HW despite extra rows).
  - mm2 batched over slot GROUPS with sum(L) <= 512 (one PSUM bank):
    pooled^T for a whole group is packed into one SBUF tile [128,6,G];
    mm2 is (4 e-chunks x 6 k) matmuls per group (72/rep vs 192), each
    streaming G columns; out is packed e-chunk-major into [128, 4G]
    bf16 and written with ONE full-rate DMA per group (3/rep).
  - Engine split (HW-measured): one-hot on DVE (GPSIMD is far slower
    per launch and per element on real HW), PSUM->SBUF copies split
    ACT/DVE/DVE, out copies alternate DVE/ACT.
  - Host gather: transpose [128e, 4ec, L] -> [L, 512] per sentence.
"""

import sys

if "/opt/trn_rl_repo" not in sys.path:
    sys.path.insert(0, "/opt/trn_rl_repo")

import numpy as np

B, S, H, E, T = 64, 512, 768, 512, 256
NCORES = 8
BL = B // NCORES  # sentences per core
KS = S // 128  # max s-tiles
KH = H // 128  # 6 h-chunks (contraction of matmul 2)
CT = T // 128  # max t-chunks of the output

_cache: dict = {}
_PARAMS = None  # (L_s, K_s) per slot, set by _set_params from input data


def _set_params(word_ids, token_lengths):
    """Slot assignment + per-slot static bounds from the actual inputs.

    Per slot: L = max len, K = max s-tiles, and per-tile column regions
    [a_k, b_k) with fresh-start points f_k. Region k covers every word id
    that any core's s-tile k contains (below its len); f_k = max(a_k,
    max_{j<k} b_j) so each column gets start=True from exactly its first
    toucher and start=False from later ones.

    Also packs slots into groups with sum(L) <= 512 (one PSUM bank) for
    the batched-mm2 path: groups = ((slot_ids, off, G, (o_s,...)), ...).
    """
    global _PARAMS
    wid = np.asarray(word_ids)
    lens = np.asarray(token_lengths).astype(np.int64)
    order = np.argsort(-lens, kind="stable")  # sentences by len desc
    slots = []
    for s in range(BL):
        grp = order[NCORES * s : NCORES * (s + 1)]
        L = int(lens[grp].max())
        Sk = [int(np.searchsorted(wid[i], lens[i])) for i in grp]
        Ks = [max(1, (s_ + 127) // 128) for s_ in Sk]
        K = max(Ks)
        P = min(128, max(1, max(Sk) - 128 * (K - 1)))
        regions = []
        prev_end = 0
        for k in range(K):
            lo, hi = T, 0
            for i, Ki, skeep in zip(grp, Ks, Sk):
                if Ki <= k:
                    continue
                lo = min(lo, int(wid[i][128 * k]))
                hi = max(hi, int(wid[i][min(128 * k + 127, skeep - 1)]) + 1)
            a = min(max(0, lo), prev_end) if k > 0 else 0
            b = min(max(hi, a), L) if k < K - 1 else L
            f = max(a, prev_end)
            regions.append((a, b, f))
            prev_end = max(prev_end, b)
        slots.append((L, K, P, tuple(regions)))

    groups, cur, cum = [], [], 0
    off = 0
    for s in range(BL):
        L = slots[s][0]
        if cur and cum + L > 512:
            groups.append((tuple(cur), off, cum, tuple(np.cumsum([0] + [slots[x][0] for x in cur])[:-1])))
            off += cum
            cur, cum = [], 0
        cur.append(s)
        cum += L
    groups.append((tuple(cur), off, cum, tuple(np.cumsum([0] + [slots[x][0] for x in cur])[:-1])))
    _PARAMS = (tuple(slots), tuple(int(x) for x in order), tuple(groups))
    return _PARAMS


def _build(reps: int = 1, bufs: int = 4, ptseng: str = "svs", outeng: str = "vs",
           dmaq: int = 2, mm2kout: int = 0, oheng: str = "g", nobands: int = 0, mm2rev: int = 0, wsrep: int = 0, dmaonly: int = 0, outdma: int = 0, slotiv: int = 0, dbg: int = 0, ablate: str = ""):
    """Build + compile the per-core Bass program for the current _PARAMS.

    ptseng: 3 chars, engines for the 3 pooled PSUM->SBUF copies
    outeng: engines for out chunk copies (cycled)
    chars: s=scalar(ACT), v=vector(DVE), g=gpsimd(Pool)
    """
    assert _PARAMS is not None, "_set_params must run before _build"
    slots = _PARAMS[0]
    ablated = set(ablate.split(",")) if ablate else set()
    from concourse import bacc, tile, mybir

    f32 = mybir.dt.float32
    bf16 = mybir.dt.bfloat16
    i32 = mybir.dt.int32
    Alu = mybir.AluOpType

    nc = bacc.Bacc("TRN2", target_bir_lowering=False, debug=False, num_devices=NCORES)

    def eng(ch):
        return {"s": nc.scalar, "v": nc.vector, "g": nc.gpsimd}[ch]

    h_d = nc.dram_tensor("h", [BL, 128, KS, H], bf16, kind="ExternalInput")
    if dbg:
        dbg_oh = nc.dram_tensor("dbg_oh", [128, KS, T], bf16, kind="ExternalOutput")
        dbg_pts = nc.dram_tensor("dbg_pts", [128, KH, T], bf16, kind="ExternalOutput")
        dbg_hs = nc.dram_tensor("dbg_hs", [128, KS, H], bf16, kind="ExternalOutput")
    ws_d = nc.dram_tensor("ws", [BL, 128, 2 * KS], f32, kind="ExternalInput")
    if wsrep:
        wsr_d = nc.dram_tensor("wsr", [128, BL, 2 * KS], f32, kind="ExternalInput")
    wt_d = nc.dram_tensor("wt", [H, E], bf16, kind="ExternalInput")  # W^T
    out_d = nc.dram_tensor("out", [BL, T, E], bf16, kind="ExternalOutput")
    if mm2rev == 2:
        outT_d = nc.dram_tensor("outT", [BL, 128, 4, T], bf16, kind="ExternalOutput")

    with tile.TileContext(nc) as tc:
        with (
            tc.tile_pool(name="const", bufs=1) as cpool,
            tc.tile_pool(name="work", bufs=bufs) as wpool,
            tc.tile_pool(name="pp", bufs=2, space="PSUM") as ppool,
            tc.tile_pool(name="po", bufs=1, space="PSUM") as opool,
        ):
            # ---- one-time constants ----
            iota_i = cpool.tile([128, T], i32)
            nc.gpsimd.iota(iota_i[:], pattern=[[1, T]], base=0, channel_multiplier=0)
            iota_b = cpool.tile([128, T], bf16)
            nc.vector.tensor_copy(iota_b[:], iota_i[:])
            wt_t = cpool.tile([128, KH, E], bf16)
            nc.sync.dma_start(wt_t[:], wt_d[:, :].rearrange("(k p) e -> p k e", p=128))

            state = {}
            wsr_t = {"t": None}

            def stage_a(it):
                """Load + one-hot + mm1 for sentence slot it%BL."""
                i = it % BL
                L, K, P, regions = slots[i]
                hs = wpool.tile([128, KS, H], bf16, tag="hs")
                h_src = h_d[i]
                full = K - 1  # tiles loaded with all 128 rows
                if dmaq >= 2 and full >= 1:
                    k2 = (full + 1) // 2
                    nc.sync.dma_start(hs[:, 0:k2, :], h_src[:, 0:k2, :])
                    if full > k2:
                        nc.scalar.dma_start(hs[:, k2:full, :], h_src[:, k2:full, :])
                    nc.scalar.dma_start(hs[0:P, full, :], h_src[0:P, full, :])
                else:
                    if full >= 1:
                        nc.sync.dma_start(hs[:, 0:full, :], h_src[:, 0:full, :])
                    nc.sync.dma_start(hs[0:P, full, :], h_src[0:P, full, :])
                if wsrep:
                    if i == 0:
                        wsr_t["t"] = wpool.tile([128, BL, 2 * KS], f32, name="wsr", tag="wsr")
                        nc.sync.dma_start(wsr_t["t"][:], wsr_d[:])
                    ws_t = wsr_t["t"][:, i, :]
                else:
                    ws_tile = wpool.tile([128, 2 * KS], f32, tag="ws")
                    nc.gpsimd.dma_start(ws_tile[:], ws_d[i])
                    ws_t = ws_tile[:]

                if dmaonly:
                    state[it] = None
                    return
                # scaled one-hot oh[s, t] = (wid==t) * rscale
                oh = wpool.tile([128, KS, T], bf16, tag="oh")
                if dbg and it == 0:
                    nc.gpsimd.memset(oh[:], 0.0)
                ohrg = [(0, L, 0)] * K if nobands else regions
                for k, (a, b, f) in enumerate(ohrg):
                    pk = P if k == K - 1 else 128
                    if b > a:
                        eng(oheng).tensor_scalar(
                            oh[0:pk, k, a:b],
                            iota_b[0:pk, a:b],
                            ws_t[0:pk, k : k + 1],
                            ws_t[0:pk, KS + k : KS + k + 1],
                            Alu.is_equal,
                            Alu.mult,
                        )

                # matmul 1: pooled^T [h, t] (already mean-scaled).
                # Region k writes cols [a,b): [a,f) accumulates onto earlier
                # tiles (start=False), [f,b) is this tile's fresh range
                # (start=True). Every column is started exactly once.
                pt_ps = [
                    ppool.tile([128, 2, T], f32, name=f"pt{j}", tag=f"pt{j}")
                    for j in range(3)
                ]
                for m in range(KH if "mm1" not in ablated else 0):
                    if nobands:
                        for k in range(K):
                            pk = P if k == K - 1 else 128
                            nc.tensor.matmul(
                                pt_ps[m // 2][:, m % 2, 0:L],
                                hs[0:pk, k, m * 128 : (m + 1) * 128],
                                oh[0:pk, k, 0:L],
                                start=(k == 0),
                                stop=(k == K - 1),
                            )
                        continue
                    for k, (a, b, f) in enumerate(regions):
                        last = k == K - 1
                        pk = P if k == K - 1 else 128
                        if f > a:
                            nc.tensor.matmul(
                                pt_ps[m // 2][:, m % 2, a:f],
                                hs[0:pk, k, m * 128 : (m + 1) * 128],
                                oh[0:pk, k, a:f],
                                start=False,
                                stop=last,
                                skip_group_check=True,
                            )
                        if b > f:
                            nc.tensor.matmul(
                                pt_ps[m // 2][:, m % 2, f:b],
                                hs[0:pk, k, m * 128 : (m + 1) * 128],
                                oh[0:pk, k, f:b],
                                start=True,
                                stop=last,
                                skip_group_check=True,
                            )
                if "mm1" in ablated:
                    for j in range(3):
                        nc.vector.memset(pt_ps[j][:], 0.5)
                if dbg and it == 0:
                    nc.sync.dma_start(dbg_oh[:], oh[:])
                    nc.sync.dma_start(dbg_hs[:], hs[:])
                state[it] = pt_ps

            def stage_b(it):
                """pooled->SBUF, mm2, out copy + DMA for sentence slot it%BL."""
                i = it % BL
                L, K, P, regions = slots[i]
                CH = (L + 127) // 128
                C = 128 * CH
                pt_ps = state.pop(it)
                if dmaonly:
                    return

                pts = wpool.tile([128, KH, T], bf16, tag="pts")
                if L < C and mm2rev != 2:
                    nc.gpsimd.memset(pts[:, :, L:C], 0.0)
                for j in range(3):
                    dst = pts[:, 2 * j : 2 * j + 2, 0:L]
                    src = pt_ps[j][:, :, 0:L]
                    ech = ptseng[j % len(ptseng)]
                    if ech == "s":
                        nc.scalar.copy(dst, src)
                    else:
                        eng(ech).tensor_copy(dst, src)

                if dbg and it == 0:
                    nc.sync.dma_start(dbg_pts[:], pts[:])

                if mm2rev == 2:
                    # reversed mm2, e-major straight to DRAM (host transposes)
                    oT = [
                        opool.tile([128, 2, T], f32, name=f"oT{j}", tag=f"oT{j}")
                        for j in range(2)
                    ]
                    for j in range(2):
                        for ec in range(2):
                            e0 = (2 * j + ec) * 128
                            for k in range(KH):
                                nc.tensor.matmul(
                                    oT[j][:, ec, 0:L],
                                    wt_t[:, k, e0 : e0 + 128],
                                    pts[:, k, 0:L],
                                    start=(k == 0),
                                    stop=(k == KH - 1),
                                )
                    outsT = wpool.tile([128, 4, T], bf16, tag="outsT")
                    for j in range(2):
                        ech = outeng[j % len(outeng)]
                        dstT = outsT[:, 2 * j : 2 * j + 2, 0:L]
                        srcT = oT[j][:, :, 0:L]
                        if ech == "s":
                            nc.scalar.copy(dstT, srcT)
                        else:
                            eng(ech).tensor_copy(dstT, srcT)
                        if outdma == 0:
                            dq = nc.sync if j == 0 else nc.scalar
                            dq.dma_start(
                                outT_d[i, :, 2 * j : 2 * j + 2, 0:L],
                                outsT[:, 2 * j : 2 * j + 2, 0:L],
                            )
                    if outdma == 1:
                        nc.gpsimd.dma_start(
                            outT_d[i, :, :, 0:L], outsT[:, :, 0:L]
                        )
                    elif outdma == 3:
                        dq = nc.sync if i % 2 else nc.scalar
                        dq.dma_start(outT_d[i, :, :, 0:L], outsT[:, :, 0:L])
                    elif outdma == 2:
                        nc.gpsimd.dma_start(
                            outT_d[i, :, 0:2, 0:L], outsT[:, 0:2, 0:L]
                        )
                        nc.vector.dma_start(
                            outT_d[i, :, 2:4, 0:L], outsT[:, 2:4, 0:L]
                        )
                    return

                if mm2rev:
                    # matmul 2 reversed: oT[e, t] = W^T-chunk.T @ pooled^T,
                    # streaming only L moving cols; transpose back to [t, e]
                    # via the DMA xbar (16x128 tiles, needs t % 128 == 0).
                    oT = [
                        opool.tile([128, 2, T], f32, name=f"oT{j}", tag=f"oT{j}")
                        for j in range(2)
                    ]
                    for j in range(2):
                        for ec in range(2):
                            e0 = (2 * j + ec) * 128
                            for k in range(KH):
                                nc.tensor.matmul(
                                    oT[j][:, ec, 0:L],
                                    wt_t[:, k, e0 : e0 + 128],
                                    pts[:, k, 0:L],
                                    start=(k == 0),
                                    stop=(k == KH - 1),
                                )
                    outsT = wpool.tile([128, 4, T], bf16, tag="outsT")
                    if L < C:
                        nc.gpsimd.memset(outsT[:, :, L:C], 0.0)
                    for j in range(2):
                        ech = ptseng[j % len(ptseng)]
                        dstT = outsT[:, 2 * j : 2 * j + 2, 0:L]
                        srcT = oT[j][:, :, 0:L]
                        if ech == "s":
                            nc.scalar.copy(dstT, srcT)
                        else:
                            eng(ech).tensor_copy(dstT, srcT)
                    outs = wpool.tile([128, CT, E], bf16, tag="outs")
                    for c in range(CH):
                        for ec in range(4):
                            dq = nc.sync if ((c * 4 + ec) % 2 == 0) else nc.scalar
                            dq.dma_start_transpose(
                                outs[:, c, ec * 128 : (ec + 1) * 128],
                                outsT[:, ec, c * 128 : (c + 1) * 128],
                            )
                        dq = nc.sync if (c % 2 == 0) else nc.scalar
                        dq.dma_start(out_d[i, c * 128 : (c + 1) * 128, :], outs[:, c, :])
                    return

                # matmul 2: out[t, e] = pooled @ W^T (k-outer so each pts
                # copy unblocks its accumulation step immediately)
                out_ps = [
                    opool.tile([128, E], f32, name=f"o2{c}", tag=f"o2{c}")
                    for c in range(CH)
                ]
                mm2iv = "mm2" not in ablated
                if not mm2iv:
                    for c in range(CH):
                        nc.vector.memset(out_ps[c][:], 0.25)
                order = (
                    [(k, c) for k in range(KH) for c in range(CH)]
                    if mm2kout
                    else [(k, c) for c in range(CH) for k in range(KH)]
                )
                for k, c in order if mm2iv else []:
                    nc.tensor.matmul(
                        out_ps[c][:],
                        pts[:, k, c * 128 : (c + 1) * 128],
                        wt_t[:, k, :],
                        start=(k == 0),
                        stop=(k == KH - 1),
                    )

                outs = wpool.tile([128, CT, E], bf16, tag="outs")
                for c in range(CH):
                    ech = outeng[c % len(outeng)]
                    if ech == "s":
                        nc.scalar.copy(outs[:, c, :], out_ps[c][:])
                    else:
                        eng(ech).tensor_copy(outs[:, c, :], out_ps[c][:])
                    dq = nc.sync if (c % 2 == 0) else nc.scalar
                    dq.dma_start(out_d[i, c * 128 : (c + 1) * 128, :], outs[:, c, :])

            if slotiv:
                seq = []
                lo, hi = 0, BL - 1
                while lo <= hi:
                    seq.append(lo)
                    if hi != lo:
                        seq.append(hi)
                    lo, hi = lo + 1, hi - 1
            else:
                seq = list(range(BL))

            def slot_of(it):
                return seq[it % BL]

            n = BL * reps
            stage_a(slot_of(0))
            for it in range(n):
                if it + 1 < n:
                    stage_a(slot_of(it + 1))
                stage_b(slot_of(it))

    nc.compile()
    return nc


def _build_v2(reps: int = 1, bufs: int = 8, ptseng: str = "svv", outeng: str = "vs",
              oheng: str = "g", mm1full: int = 0, gpbufs: int = 3, wseng: str = "g",
              hwloop: int = 0, ablate: str = "", hsmode: int = 0, hsq: int = 2,
              gmm1: int = 0, outq: int = 0):
    """Grouped-mm2 builder: fewer DMAs, 72 mm2 matmuls, packed out DMA.

    - hidden: one DMA for full s-tiles + one for the partial tile (2/slot).
    - mm2 batched over slot groups with sum(L) <= 512: pooled^T for a whole
      group lives in one SBUF tile [128, 6, G]; mm2 runs (4 e-chunks x 6 k)
      per group into a single-bank PSUM tile [128, G].
    - out: per group, 4 e-chunk copies pack into [128, 4G] bf16, one DMA.
    - mm1full=1: first s-tile sweeps [0, L) fresh so later tiles need one
      accumulate piece each (fewer PE instructions, slightly more rows).
    """
    assert _PARAMS is not None, "_set_params must run before _build_v2"
    slots, _, groups = _PARAMS
    TOT = sum(g[2] for g in groups)
    abl = set(ablate.split(",")) if ablate else set()
    from concourse import bacc, tile, mybir

    f32 = mybir.dt.float32
    bf16 = mybir.dt.bfloat16
    i32 = mybir.dt.int32
    Alu = mybir.AluOpType

    nc = bacc.Bacc("TRN2", target_bir_lowering=False, debug=False, num_devices=NCORES)

    def eng(ch):
        return {"s": nc.scalar, "v": nc.vector, "g": nc.gpsimd}[ch]

    def copy_to(ch, dst, src):
        if ch == "s":
            nc.scalar.copy(dst, src)
        else:
            eng(ch).tensor_copy(dst, src)

    h_d = nc.dram_tensor("h", [BL, 128, KS, H], bf16, kind="ExternalInput")
    ws_d = nc.dram_tensor("ws", [BL, 128, 2 * KS], f32, kind="ExternalInput")
    wsr_d = nc.dram_tensor("wsr", [128, BL, 2 * KS], f32, kind="ExternalInput")
    wt_d = nc.dram_tensor("wt", [H, E], bf16, kind="ExternalInput")  # W^T
    outP_d = nc.dram_tensor("outP", [128, 4 * TOT], bf16, kind="ExternalOutput")

    with tile.TileContext(nc) as tc:
        with (
            tc.tile_pool(name="const", bufs=1) as cpool,
            tc.tile_pool(name="work", bufs=bufs) as wpool,
            tc.tile_pool(name="grp", bufs=gpbufs) as gpool,
            tc.tile_pool(name="pp", bufs=(1 if gmm1 else 2), space="PSUM") as ppool,
            tc.tile_pool(name="po", bufs=1, space="PSUM") as opool,
        ):
            # ---- one-time constants ----
            iota_i = cpool.tile([128, T], i32)
            nc.gpsimd.iota(iota_i[:], pattern=[[1, T]], base=0, channel_multiplier=0)
            iota_b = cpool.tile([128, T], bf16)
            nc.vector.tensor_copy(iota_b[:], iota_i[:])
            wt_t = cpool.tile([128, KH, E], bf16)
            nc.sync.dma_start(wt_t[:], wt_d[:, :].rearrange("(k p) e -> p k e", p=128))

            state: dict = {}

            if "mm2only" in abl:
                # PE p-state probe: resident pooled tiles, mm2 stream only.
                for gi in range(len(groups)):
                    t = cpool.tile([128, KH, 512], bf16, name=f"ptsc{gi}", tag=f"ptsc{gi}")
                    nc.vector.memset(t[:], 0.25)
                    state[("ptsc", gi)] = t

            def stage_a(i, gi, o_s, G, hq, last_in_group=False):
                """Load + one-hot + mm1 + pts copy for slot i (group gi)."""
                if "mm2only" in abl:
                    return
                L, K, P, regions = slots[i]
                hs = wpool.tile([128, KS, H], bf16, tag="hs")
                h_src = h_d[i]
                hq_ring = {2: [nc.sync, nc.scalar],
                           3: [nc.sync, nc.scalar, nc.gpsimd],
                           4: [nc.sync, nc.scalar, nc.vector]}[hsq]
                dq = hq_ring[hq % len(hq_ring)]
                dq2 = hq_ring[(hq + 1) % len(hq_ring)]
                if "hsdma" not in abl:
                    if hsmode == 2:
                        dq.dma_start(hs[:, :, :], h_src[:, :, :])
                    elif hsmode == 1:
                        dq.dma_start(hs[:, 0:K, :], h_src[:, 0:K, :])
                    else:
                        if K > 1:
                            dq.dma_start(hs[:, 0 : K - 1, :], h_src[:, 0 : K - 1, :])
                        dq2.dma_start(hs[0:P, K - 1, :], h_src[0:P, K - 1, :])
                if i == 0:
                    state["wsr"] = wpool.tile([128, BL, 2 * KS], f32, name="wsr", tag="wsr")
                    if "wsdma" not in abl:
                        eng(wseng).dma_start(state["wsr"][:], wsr_d[:])
                ws_t = state["wsr"][:, i, :]

                # scaled one-hot oh[s, t] = (wid==t) * rscale
                oh = wpool.tile([128, KS, T], bf16, tag="oh")
                ohrg = [((0, L, 0) if (mm1full and k == 0) else r)
                        for k, r in enumerate(regions)]
                for k, (a, b, f) in enumerate(ohrg):
                    pk = P if k == K - 1 else 128
                    if b > a and "oh" not in abl:
                        eng(oheng).tensor_scalar(
                            oh[0:pk, k, a:b],
                            iota_b[0:pk, a:b],
                            ws_t[0:pk, k : k + 1],
                            ws_t[0:pk, KS + k : KS + k + 1],
                            Alu.is_equal,
                            Alu.mult,
                        )

                # matmul 1: pooled^T [h, t] (mean-scaled via rscale)
                if o_s == 0:
                    state[("pts", gi)] = gpool.tile([128, KH, 512], bf16, name="ptsg", tag="ptsg")
                pts_g = state[("pts", gi)]

                if gmm1:
                    # accumulate straight into the per-group PSUM tile
                    if o_s == 0:
                        state[("ptG", gi)] = ppool.tile(
                            [128, KH, 512], f32, name="ptG", tag="ptG")
                    ptg = state[("ptG", gi)]

                    def mm1_out(m, c0, c1):
                        return ptg[:, m, o_s + c0 : o_s + c1]
                else:
                    pt_ps = [
                        ppool.tile([128, 2, T], f32, name=f"pt{j}", tag=f"pt{j}")
                        for j in range(3)
                    ]

                    def mm1_out(m, c0, c1):
                        return pt_ps[m // 2][:, m % 2, c0:c1]

                for m in range(KH if "mm1" not in abl else 0):
                    for k, (a, b, f) in enumerate(ohrg):
                        last = k == K - 1
                        pk = P if k == K - 1 else 128
                        if mm1full:
                            nc.tensor.matmul(
                                mm1_out(m, a, b),
                                hs[0:pk, k, m * 128 : (m + 1) * 128],
                                oh[0:pk, k, a:b],
                                start=(k == 0),
                                stop=last,
                                skip_group_check=True,
                            )
                            continue
                        if f > a:
                            nc.tensor.matmul(
                                mm1_out(m, a, f),
                                hs[0:pk, k, m * 128 : (m + 1) * 128],
                                oh[0:pk, k, a:f],
                                start=False,
                                stop=last,
                                skip_group_check=True,
                            )
                        if b > f:
                            nc.tensor.matmul(
                                mm1_out(m, f, b),
                                hs[0:pk, k, m * 128 : (m + 1) * 128],
                                oh[0:pk, k, f:b],
                                start=True,
                                stop=last,
                                skip_group_check=True,
                            )
                    if (gmm1 and last_in_group and m % 2 == 1
                            and "ptscopy" not in abl and "mm1" not in abl):
                        j = m // 2
                        copy_to(ptseng[j % len(ptseng)],
                                pts_g[:, 2 * j : 2 * j + 2, 0:G],
                                state[("ptG", gi)][:, 2 * j : 2 * j + 2, 0:G])

                # pooled^T -> group SBUF tile (f32 -> bf16), 3 copies
                if not gmm1:
                    for j in range(3 if "ptscopy" not in abl else 0):
                        copy_to(ptseng[j % len(ptseng)],
                                pts_g[:, 2 * j : 2 * j + 2, o_s : o_s + L],
                                pt_ps[j][:, :, 0:L])

            def stage_b(gi, off, G, oq):
                """Batched mm2 + packed out copy + one DMA for group gi."""
                if "mm2only" in abl:
                    pts_g = state[("ptsc", gi)]
                else:
                    pts_g = state.pop(("pts", gi))
                outs = gpool.tile([128, 4 * 512], bf16, tag="outsg")
                for ec in range(4):
                    oT = opool.tile([128, 512], f32, name=f"oT{ec % 2}", tag=f"oT{ec % 2}")
                    for k in range(KH if "mm2" not in abl else 0):
                        nc.tensor.matmul(
                            oT[:, 0:G],
                            wt_t[:, k, ec * 128 : (ec + 1) * 128],
                            pts_g[:, k, 0:G],
                            start=(k == 0),
                            stop=(k == KH - 1),
                        )
                    if "outcopy" not in abl:
                        copy_to(outeng[ec % len(outeng)],
                                outs[:, ec * G : (ec + 1) * G], oT[:, 0:G])
                if "outdma" not in abl:
                    if outq == 1:
                        nc.gpsimd.dma_start(
                            outP_d[:, 4 * off : 4 * off + 4 * G], outs[:, 0 : 4 * G])
                    elif outq == 2:
                        # split halves across both HWDGE queues
                        nc.sync.dma_start(
                            outP_d[:, 4 * off : 4 * off + 2 * G], outs[:, 0 : 2 * G])
                        nc.scalar.dma_start(
                            outP_d[:, 4 * off + 2 * G : 4 * off + 4 * G],
                            outs[:, 2 * G : 4 * G])
                    else:
                        dq = nc.sync if oq % 2 == 0 else nc.scalar
                        dq.dma_start(
                            outP_d[:, 4 * off : 4 * off + 4 * G], outs[:, 0 : 4 * G])

            NG = len(groups)
            qs = {"hq": 0, "oq": 0}

            def emit_reps():
                for r in range(reps):
                    for gi, (gslots, off, G, offs) in enumerate(groups):
                        for si, s in enumerate(gslots):
                            stage_a(s, gi, int(offs[si]), G, qs["hq"],
                                    last_in_group=(si == len(gslots) - 1))
                            qs["hq"] += 1
                        if gi >= 1:
                            pg = groups[gi - 1]
                            stage_b(gi - 1, pg[1], pg[2], qs["oq"])
                            qs["oq"] += 1
                    lg = groups[NG - 1]
                    stage_b(NG - 1, lg[1], lg[2], qs["oq"])
                    qs["oq"] += 1

            if hwloop:
                with tc.For_i(0, hwloop) as _i:
                    emit_reps()
            else:
                emit_reps()

    nc.compile()
    return nc


def _get_nc(reps: int = 1, **opts):
    key = f"nc{reps}|{sorted(opts.items())}|{hash(_PARAMS)}"
    if key not in _cache:
        o = dict(opts)
        if o.pop("v2", 0):
            _cache[key] = _build_v2(reps, **o)
        else:
            _cache[key] = _build(reps, **o)
    return _cache[key]


def _in_maps(hidden_states, word_ids, token_lengths, W, b):
    """Per-core input maps (also sets _PARAMS / slot assignment)."""
    import ml_dtypes

    bf16 = ml_dtypes.bfloat16
    if _PARAMS is None:
        _set_params(word_ids, token_lengths)
    order = _PARAMS[1]
    wid = np.asarray(word_ids)
    lens = np.asarray(token_lengths)

    wt = np.ascontiguousarray(W.T).astype(bf16)
    maps = [dict(wt=wt) for _ in range(NCORES)]
    h_all = np.empty((NCORES, BL, 128, KS, H), dtype=bf16)
    ws_all = np.zeros((NCORES, BL, 128, 2 * KS), dtype=np.float32)
    for s in range(BL):
        for c in range(NCORES):
            i = order[NCORES * s + c]
            h_all[c, s] = (
                hidden_states[i].reshape(KS, 128, H).transpose(1, 0, 2).astype(bf16)
            )
            cnt = np.bincount(wid[i], minlength=T).astype(np.float32)
            rs = np.where(wid[i] < lens[i], 1.0 / np.maximum(cnt[wid[i]], 1.0), 0.0)
            ws_all[c, s, :, 0:KS] = wid[i].reshape(KS, 128).T
            ws_all[c, s, :, KS : 2 * KS] = rs.reshape(KS, 128).T
    for c in range(NCORES):
        maps[c]["h"] = np.ascontiguousarray(h_all[c])
        maps[c]["ws"] = np.ascontiguousarray(ws_all[c])
        maps[c]["wsr"] = np.ascontiguousarray(ws_all[c].transpose(1, 0, 2))
    return maps


DEFAULT_OPTS: dict = {"v2": 1, "oheng": "v", "mm1full": 1}


def kernel(hidden_states, word_ids, token_lengths, W, b):
    from concourse import bass_utils

    _set_params(word_ids, token_lengths)
    maps = _in_maps(hidden_states, word_ids, token_lengths, W, b)
    nc = _get_nc(1, **DEFAULT_OPTS)
    res = bass_utils.run_bass_kernel_spmd(nc, maps, core_ids=list(range(NCORES)))

    slots, order = _PARAMS[0], _PARAMS[1]
    out = np.empty((B, T, E), dtype=np.float32)
    bia = b.astype(np.float32)[None, :]
    if DEFAULT_OPTS.get("v2"):
        groups = _PARAMS[2]
        for c in range(NCORES):
            outP = np.asarray(res.results[c]["outP"]).astype(np.float32)
            for gslots, off, G, offs in groups:
                arr = outP[:, 4 * off : 4 * off + 4 * G].reshape(128, 4, G)
                for si, s in enumerate(gslots):
                    L = slots[s][0]
                    o = int(offs[si])
                    i = order[NCORES * s + c]
                    blk = np.zeros((T, E), np.float32)
                    blk[:L] = arr[:, :, o : o + L].transpose(2, 1, 0).reshape(L, E)
                    blk += bia
                    out[i] = blk
        return out
    rev2 = DEFAULT_OPTS.get("mm2rev") == 2
    for s in range(BL):
        L = slots[s][0]
        CH = (L + 127) // 128
        for c in range(NCORES):
            if rev2:
                i = order[NCORES * s + c]
                oT = np.asarray(res.results[c]["outT"][s]).astype(np.float32)
                blk = np.zeros((T, E), np.float32)
                blk[:L] = oT.transpose(2, 1, 0).reshape(T, E)[:L]
                blk += bia
                out[i] = blk
                continue
            i = order[NCORES * s + c]
            blk = np.asarray(res.results[c]["out"][s]).astype(np.float32)
            blk[: 128 * CH] += bia
            blk[128 * CH :] = bia  # rows never touched on device
            out[i] = blk
    return out

